# revision 36
# baseline (speedup 1.0000x reference)
"""Multi-head attention (B=8, S=1024, D=1024, H=16) on 8 trn2 NeuronCores, v3.

Batch-parallel (1 batch/core), zero collectives. Per core:
  - k-proj (bf16, et-major single pass) -> kT resident in SBUF
  - v-proj (bf16) in 4 head-quarter passes; quarters 2,3 interleaved into
    early attention beats
  - attention per head-pair pr, sh-outer beats: row-packed score matmuls
    (two K=64 matmuls in row groups 0-1 / 2-3 run concurrently), one exp
    [128,1024] per beat on ScalarE, av accumulation [65,512] per (j, sh)
  - av evacuated to SBUF immediately (psum freed); softmax denominators
    batched per pr: 4 rows -> DRAM -> spread [128,16] -> one reciprocal ->
    DRAM -> partition-broadcast loads -> DVE scale into catT
  - out-proj (bf16 cat x bf16 wo)
PSUM: big 3x[128,1024] (6 banks) + av0/av1 [65,512] (1 bank each) = 8 banks.
DMA: nc.sync = critical path (xk, wk, wq, smalls), nc.scalar = prefetch
(xq, wv, xv, wo[0:3]).
"""

import sys

if "/opt/trn_rl_repo" not in sys.path:
    sys.path.insert(0, "/opt/trn_rl_repo")

import numpy as np

B, S, D, H = 8, 1024, 1024, 16
Dh = D // H  # 64
P = 128
NT = 8
SH = 512

_CACHE = {}


def _prep_x(x):
    # x [S, D] -> [2, 128, 4096]; out[hf, p, k*512 + s'] = x[hf*512+s', k*128+p]
    return np.ascontiguousarray(x.reshape(2, SH, NT, P).transpose(0, 3, 2, 1)).reshape(
        2, P, NT * SH
    )


def _prep_w(Wcat):
    # W [out 1024, in 1024] -> [8, 128, 1024]; out[ot, p, k*128+oc] = W[ot*128+oc, k*128+p]
    return np.ascontiguousarray(Wcat.reshape(NT, P, NT, P).transpose(0, 3, 2, 1)).reshape(
        NT, P, NT * P
    )


def _prep_wv(Wvcat):
    # rhs layout [8, 128, 1024]; out[k, p, e] = Wv_cat[e, k*128+p]
    return np.ascontiguousarray(Wvcat.T.reshape(NT, P, D))


def _prep_bias(b):
    # [1024] -> [128, 8]; out[p, i] = b[i*128+p]
    return np.ascontiguousarray(b.reshape(NT, P).T)


def _bf16(a):
    import ml_dtypes

    return np.asarray(a).astype(ml_dtypes.bfloat16)


def _build():
    import concourse.mybir as mybir
    import concourse.tile as tile
    from concourse import bacc

    dt = mybir.dt
    f32 = dt.float32
    bf16 = dt.bfloat16
    AF = mybir.ActivationFunctionType

    nc = bacc.Bacc(None, target_bir_lowering=False)

    with tile.TileContext(nc) as tc:
        with (
            tc.tile_pool(name="dram", bufs=1, space="DRAM") as dram,
            tc.tile_pool(name="consts", bufs=1) as consts,
            tc.tile_pool(name="xq_p", bufs=1) as xq_p,
            tc.tile_pool(name="xh_p", bufs=2) as xh_p,
            tc.tile_pool(name="wv_p", bufs=2) as wv_p,
            tc.tile_pool(name="wst_p", bufs=3) as wst_p,
            tc.tile_pool(name="kt_p", bufs=1) as kt_p,
            tc.tile_pool(name="vaug_p", bufs=1) as vaug_p,
            tc.tile_pool(name="cat_p", bufs=1) as cat_p,
            tc.tile_pool(name="qp_p", bufs=2) as qp_p,
            tc.tile_pool(name="ex_p", bufs=6) as ex_p,
            tc.tile_pool(name="avst_p", bufs=8) as avst_p,
            tc.tile_pool(name="spr_p", bufs=2) as spr_p,
            tc.tile_pool(name="bcrc_p", bufs=4) as bcrc_p,
            tc.tile_pool(name="tm_p", bufs=2) as tm_p,
            tc.tile_pool(name="st_p", bufs=2) as st_p,
            tc.tile_pool(name="ps", bufs=1, space="PSUM") as ps_p,
        ):
            # ---- DRAM I/O ----
            xq = dram.tile([2, P, NT * SH], bf16, kind="ExternalInput", name="xq", uniquify=False)
            xk = dram.tile([2, P, NT * SH], bf16, kind="ExternalInput", name="xk", uniquify=False)
            xv = dram.tile([2, P, NT * SH], bf16, kind="ExternalInput", name="xv", uniquify=False)
            wq = dram.tile([NT, P, D], bf16, kind="ExternalInput", name="wq", uniquify=False)
            wk = dram.tile([NT, P, D], bf16, kind="ExternalInput", name="wk", uniquify=False)
            wv = dram.tile([NT, P, D], bf16, kind="ExternalInput", name="wv", uniquify=False)
            wo = dram.tile([NT, P, D], bf16, kind="ExternalInput", name="wo", uniquify=False)
            bqd = dram.tile([P, NT], f32, kind="ExternalInput", name="bqd", uniquify=False)
            bkd = dram.tile([P, NT], f32, kind="ExternalInput", name="bkd", uniquify=False)
            bod = dram.tile([P, NT], f32, kind="ExternalInput", name="bod", uniquify=False)
            outT = dram.tile([NT, P, S], bf16, kind="ExternalOutput", name="outT", uniquify=False)

            # ---- consts + persistent SBUF ----
            bq_sb = consts.tile([P, NT], f32, name="bq_sb")
            bk_sb = consts.tile([P, NT], f32, name="bk_sb")
            bo_sb = consts.tile([P, NT], f32, name="bo_sb")
            nc.sync.dma_start(bq_sb[:], bqd[:])
            nc.sync.dma_start(bk_sb[:], bkd[:])
            nc.sync.dma_start(bo_sb[:], bod[:])

            kT = kt_p.tile([P, NT, S], bf16, name="kT")
            catT = cat_p.tile([P, NT, S], bf16, name="catT")
            v_aug = vaug_p.tile([P, NT, H, Dh + 1], bf16, name="v_aug")
            nc.vector.memset(v_aug[:, :, :, Dh], 1.0)

            # dual-queue startup: qSP carries xk0 + odd wk + xq; qAct carries
            # xk1 + even wk + wv + xv. Both queues feed k-proj from t~0.
            wk_tiles = [
                wst_p.tile([P, D], bf16, name="w", tag="w", bufs=NT)
                for _ in range(NT)
            ]
            nc.sync.dma_start(wk_tiles[0][:], wk[0])
            xk0 = xh_p.tile([P, NT * SH], bf16, name="xk0", tag="xh")
            xk1 = xh_p.tile([P, NT * SH], bf16, name="xk1", tag="xh")
            for c in range(8):
                nc.sync.dma_start(
                    xk0[:, c * 512 : (c + 1) * 512], xk[0][:, c * 512 : (c + 1) * 512]
                )
                nc.scalar.dma_start(
                    xk1[:, c * 512 : (c + 1) * 512], xk[1][:, c * 512 : (c + 1) * 512]
                )
            for et in range(1, NT):
                eng = nc.sync if et % 2 else nc.scalar
                eng.dma_start(wk_tiles[et][:], wk[et])
            xq_sb = xq_p.tile([P, 2 * NT * SH], bf16, name="xq_sb")
            for c in range(8):
                eng = nc.sync if c % 2 == 0 else nc.scalar
                eng.dma_start(
                    xq_sb[:, c * 1024 : (c + 1) * 1024],
                    xq[c // 4][:, (c % 4) * 1024 : (c % 4 + 1) * 1024],
                )
            wv_sb = [
                wv_p.tile([P, NT * SH], bf16, name=f"wv{eh}", tag="wv")
                for eh in range(2)
            ]
            for k in range(NT):
                nc.scalar.dma_start(
                    wv_sb[0][:, k * SH : (k + 1) * SH], wv[k][:, 0:SH]
                )
            xv0 = xh_p.tile([P, NT * SH], bf16, name="xv0", tag="xh")
            xv1 = xh_p.tile([P, NT * SH], bf16, name="xv1", tag="xh")
            for c in range(8):
                xvt = (xv0, xv1)[c // 4]
                eng = nc.scalar if c % 2 == 0 else nc.sync
                eng.dma_start(
                    xvt[:, (c % 4) * 1024 : (c % 4 + 1) * 1024],
                    xv[c // 4][:, (c % 4) * 1024 : (c % 4 + 1) * 1024],
                )
            for k in range(NT):
                nc.scalar.dma_start(
                    wv_sb[1][:, k * SH : (k + 1) * SH], wv[k][:, SH:D]
                )

            def big_slot():
                return ps_p.tile([P, S], f32, name="bg", tag="big", bufs=3)

            # HAM warmup: PE clock-gate releases only after ~3.4us of sustained
            # matmul activity; without this the whole k-proj runs at 1.2 GHz.
            warm_sb = consts.tile([P, P], bf16, name="warm_sb")
            nc.vector.memset(warm_sb[:], 0.0)

            def emit_warm(n, skip_check=False):
                wps = ps_p.tile([P, SH], f32, name="wps", tag="av0", bufs=1)
                for _ in range(n):
                    nc.tensor.matmul(
                        wps[:, 0:P], warm_sb[:], warm_sb[:],
                        skip_group_check=skip_check,
                    )

            emit_warm(48)

            # ---- k-projection, et-major single pass ----
            # DMA-starve stalls inside the early et groups (xk/wk chunks
            # still landing, consistently ~6.7us in et0 + ~3us in et1-3
            # across runs) exceed the HAM MID window and re-throttle the PE
            # to 1.2 GHz for the rest of k-proj. Interleave data-free warm
            # matmuls INTO the early groups so the PE never idles long
            # enough to re-throttle; they run only while real MMs wait.
            kfill = {0: 6, 1: 4, 2: 2}
            xkh = (xk0, xk1)
            for et in range(NT):
                w = wk_tiles[et]
                slot = big_slot()
                for hf in range(2):
                    for k in range(NT):
                        nc.tensor.matmul(
                            slot[:, hf * SH : (hf + 1) * SH],
                            w[:, k * P : (k + 1) * P],
                            xkh[hf][:, k * SH : (k + 1) * SH],
                            start=(k == 0),
                            stop=(k == NT - 1),
                        )
                        if k % 2 == 1 and k < NT - 1:
                            emit_warm(kfill.get(et, 0), skip_check=True)
                nc.vector.tensor_scalar_add(
                    kT[:, et, :], slot[:], bk_sb[:, et : et + 1]
                )


            # ---- v-projection quarter passes: heads 4q..4q+3 ----
            xvh = (xv0, xv1)

            def emit_vq_chunk(q, tt0, ntt=4):
                eh, qh = divmod(q, 2)
                slot = big_slot()
                for sub in range(ntt):
                    tt = tt0 + sub
                    hf, tl = divmod(tt, 4)
                    for k in range(NT):
                        nc.tensor.matmul(
                            slot[:, sub * 256 : (sub + 1) * 256],
                            xvh[hf][:, k * SH + tl * P : k * SH + (tl + 1) * P],
                            wv_sb[eh][:, k * SH + qh * 256 : k * SH + (qh + 1) * 256],
                            start=(k == 0),
                            stop=(k == NT - 1),
                        )
                    nc.vector.tensor_copy(
                        v_aug[:, tt, q * 4 : (q + 1) * 4, 0:Dh],
                        slot[:, sub * 256 : (sub + 1) * 256].rearrange(
                            "p (g c) -> p g c", c=Dh
                        ),
                    )

            emit_warm(8)
            emit_vq_chunk(0, 0)
            emit_warm(8)
            emit_vq_chunk(0, 4)
            emit_warm(8)
            vq_todo = [(q, tt0) for q in (1, 2, 3) for tt0 in (0, 2, 4, 6)]

            # ---- fused q-projection + attention ----
            def emit_qproj_chunk(pr, idx, qst):
                # 4 matmuls per call; two calls share one psum group so each
                # insert stays under the per-beat tensor slack
                if idx == 0:
                    wqt = wst_p.tile([P, D], bf16, name="wqt", tag="wq", bufs=2)
                    nc.sync.dma_start(wqt[:], wq[pr])
                    qp_t = qp_p.tile([P, S], bf16, name="qp", tag="qp")
                    qst[pr] = [wqt, qp_t, None]
                wqt, qp_t, slot = qst[pr]
                half, kc = divmod(idx, 2)
                hf, ch = divmod(half, 2)
                if kc == 0:
                    slot = big_slot()
                    qst[pr][2] = slot
                for k in range(4 * kc, 4 * kc + 4):
                    nc.tensor.matmul(
                        slot[:, 0:256],
                        wqt[:, k * P : (k + 1) * P],
                        xq_sb[
                            :,
                            hf * 4096 + k * SH + ch * 256 : hf * 4096
                            + k * SH
                            + (ch + 1) * 256,
                        ],
                        start=(k == 0),
                        stop=(k == NT - 1),
                    )
                if kc == 1:
                    nc.vector.tensor_scalar_add(
                        qp_t[:, half * 256 : (half + 1) * 256],
                        slot[:, 0:256],
                        bq_sb[:, pr : pr + 1],
                    )
                return qp_t

            # prefetch all out-projection weights now: qSP is quiet during
            # attention and the slots are fresh (no blocking waits)
            wots = []
            for ft in range(NT):
                wot = wst_p.tile([P, D], bf16, name="wot", tag="wob", bufs=NT)
                nc.sync.dma_start(wot[:], wo[ft])
                wots.append(wot)

            def emit_oproj(ft, sh):
                wot = wots[ft]
                slot = big_slot()
                for et in range(NT):
                    nc.tensor.matmul(
                        slot[:, 0:SH],
                        wot[:, et * P : (et + 1) * P],
                        catT[:, et, sh * SH : (sh + 1) * SH],
                        start=(et == 0),
                        stop=(et == NT - 1),
                    )
                st = st_p.tile([P, SH], bf16, name="st", tag="st", bufs=4)
                nc.vector.tensor_scalar_add(
                    st[:], slot[:, 0:SH], bo_sb[:, ft : ft + 1]
                )
                nc.scalar.dma_start(outT[ft][:, sh * SH : (sh + 1) * SH], st[:])

            qst = {}
            qps = {}
            for idx in range(8):
                qps[0] = emit_qproj_chunk(0, idx, qst)
                if idx in (1, 3, 5):
                    emit_warm(6)

            def emit_sc(pr, qp_t, beat):
                sh, tt = divmod(beat, NT)
                sc = big_slot()
                nc.tensor.matmul(
                    sc[:, 0:SH],
                    kT[0:Dh, pr, tt * P : (tt + 1) * P],
                    qp_t[0:Dh, sh * SH : (sh + 1) * SH],
                )
                nc.tensor.matmul(
                    sc[:, SH:S],
                    kT[Dh:P, pr, tt * P : (tt + 1) * P],
                    qp_t[Dh:P, sh * SH : (sh + 1) * SH],
                )
                ex_t = ex_p.tile([P, S], bf16, name="ex", tag="ex")
                nc.scalar.activation(ex_t[:], sc[:], AF.Exp, scale=0.125)
                return ex_t

            def emit_av(pr, stt, beat, ex_t):
                sh, tt = divmod(beat, NT)
                avs, avsts = stt["avs"], stt["avsts"]
                if tt == 0:
                    for j in range(2):
                        avs[(j, sh)] = ps_p.tile(
                            [Dh + 1, SH], f32, name=f"av{j}", tag=f"av{j}", bufs=1
                        )
                for j in range(2):
                    nc.tensor.matmul(
                        avs[(j, sh)][:],
                        v_aug[:, tt, 2 * pr + j, :],
                        ex_t[:, j * SH : (j + 1) * SH],
                        start=(tt == 0),
                        stop=(tt == NT - 1),
                    )
                if tt == NT - 1:
                    # evacuate to SBUF promptly so the psum bank frees
                    for j in range(2):
                        av = avs.pop((j, sh))
                        avt = avst_p.tile(
                            [Dh + 1, SH], f32, name="avst", tag="avst"
                        )
                        nc.vector.tensor_copy(avt[:], av[:])
                        avsts[(j, sh)] = avt

            def emit_norm(pr, stt, sh):
                avsts = stt["avsts"]
                for j in range(2):
                    avt = avsts.pop((j, sh))
                    # denominator row -> partition 0 (DMA shifts partitions),
                    # broadcast on the idle GpSimd engine, one fast reciprocal
                    # + scale on DVE -- no DRAM round trips
                    dn0 = spr_p.tile([1, SH], f32, name="dn0", tag="spr", bufs=4)
                    nc.sync.dma_start(dn0[0:1, :], avt[Dh : Dh + 1, :])
                    bcrc = bcrc_p.tile([Dh, SH], f32, name="bcrc", tag="bcrc")
                    nc.gpsimd.partition_broadcast(bcrc[:], dn0[0:1, :])
                    nc.vector.reciprocal_approx_fast(bcrc[:], bcrc[:])
                    if j == 0:
                        nc.vector.tensor_mul(
                            catT[0:Dh, pr, sh * SH : (sh + 1) * SH],
                            avt[0:Dh, :],
                            bcrc[:],
                        )
                    else:
                        tm_t = tm_p.tile([Dh, SH], bf16, name="tm", tag="tm")
                        nc.vector.tensor_mul(tm_t[:], avt[0:Dh, :], bcrc[:])
                        nc.sync.dma_start(
                            catT[Dh:P, pr, sh * SH : (sh + 1) * SH], tm_t[:]
                        )

            # seamless beat stream across all head pairs: ACT never sees a
            # pr boundary; av work lags LAG beats behind the score stream
            state = {}
            exs = {}
            LAG = 3
            TOT = NT * 16
            for gb in range(TOT + LAG):
                # AV pair first: following the previous beat's full-row MMs
                # its LDW backgrounds cleanly (216ns slot); emitted after the
                # score pair it pays ~+120ns waiting out the delayed second
                # score MM. The score pair's own first LDW backgrounds fine
                # during AV streams.
                ab = gb - LAG
                if ab >= 0:
                    apr, abeat = divmod(ab, 16)
                    stt = state[apr]
                    emit_av(apr, stt, abeat, exs.pop(ab))
                    if abeat == 7:
                        emit_norm(apr, stt, 0)
                    elif abeat == 15:
                        emit_norm(apr, stt, 1)
                        del state[apr]
                if gb < TOT:
                    pr, beat = divmod(gb, 16)
                    if beat == 0:
                        state[pr] = {"qp": qps.pop(pr), "avs": {}, "avsts": {}}
                    exs[gb] = emit_sc(pr, state[pr]["qp"], beat)
                    if beat % 2 == 0 and pr + 1 < NT:
                        qps[pr + 1] = emit_qproj_chunk(pr + 1, beat // 2, qst)
                    if pr == NT - 1 and beat in (11, 13, 15):
                        emit_oproj((beat - 11) // 2, 0)
                    if pr < 6 and beat in (5, 13) and vq_todo:
                        emit_vq_chunk(*vq_todo.pop(0), ntt=2)

            # ---- output projection, sh-major: sh0 needs no waits, sh1's
            # et7 dependency (pr7 norm) resolves while sh0 runs ----
            for sh in range(2):
                for ft in range(NT):
                    if sh == 0 and ft < 3:
                        continue
                    wot = wots[ft]
                    slot = big_slot()
                    for et in range(NT):
                        nc.tensor.matmul(
                            slot[:, 0:SH],
                            wot[:, et * P : (et + 1) * P],
                            catT[:, et, sh * SH : (sh + 1) * SH],
                            start=(et == 0),
                            stop=(et == NT - 1),
                        )
                    st = st_p.tile([P, SH], bf16, name="st", tag="st", bufs=4)
                    nc.vector.tensor_scalar_add(
                        st[:], slot[:, 0:SH], bo_sb[:, ft : ft + 1]
                    )
                    nc.scalar.dma_start(outT[ft][:, sh * SH : (sh + 1) * SH], st[:])

    nc.compile()
    return nc


def make_in_maps(query, key, value, Wq, Wk, Wv, Wo, bq, bk, bv, bo):
    query = np.asarray(query, np.float32)
    key = np.asarray(key, np.float32)
    value = np.asarray(value, np.float32)
    Wq_c = np.asarray(Wq, np.float32).reshape(D, D)
    Wk_c = np.asarray(Wk, np.float32).reshape(D, D)
    Wv_c = np.asarray(Wv, np.float32).reshape(D, D)
    Wo_c = np.asarray(Wo, np.float32)
    bq_c = np.asarray(bq, np.float32).reshape(D)
    bk_c = np.asarray(bk, np.float32).reshape(D)
    bv_c = np.asarray(bv, np.float32).reshape(D)
    bo_c = np.asarray(bo, np.float32)

    import ml_dtypes

    shared = {
        "wq": _bf16(_prep_w(Wq_c)),
        "wk": _bf16(_prep_w(Wk_c)),
        "wv": _bf16(_prep_wv(Wv_c)),
        "wo": _bf16(_prep_w(Wo_c)),
        "bqd": _prep_bias(bq_c),
        "bkd": _prep_bias(bk_c),
        # attn rows sum to 1: attn @ (v + bv) = attn @ v + bv; bv then flows
        # through the output projection as an extra bias Wo @ bv.
        "bod": _prep_bias(bo_c + Wo_c @ bv_c),
    }
    in_maps = []
    for b in range(B):
        m = dict(shared)
        m["xq"] = _bf16(_prep_x(query[b]))
        m["xk"] = _bf16(_prep_x(key[b]))
        m["xv"] = _bf16(_prep_x(value[b]))
        in_maps.append(m)
    return in_maps


def kernel(query, key, value, mask, Wq, bq, Wk, bk, Wv, bv, Wo, bo):
    from concourse.bass_utils import run_bass_kernel_spmd

    if "nc" not in _CACHE:
        _CACHE["nc"] = _build()
    nc = _CACHE["nc"]

    in_maps = make_in_maps(query, key, value, Wq, Wk, Wv, Wo, bq, bk, bv, bo)
    res = run_bass_kernel_spmd(nc, in_maps, core_ids=list(range(B)))
    out = np.empty((B, S, D), np.float32)
    for b in range(B):
        out[b] = np.asarray(res.results[b]["outT"], np.float32).reshape(D, S).T
    return out



# revision 37
# speedup vs baseline: 1.0432x; 1.0432x over previous
"""Multi-head attention (B=8, S=1024, D=1024, H=16) on 8 trn2 NeuronCores, v3.

Batch-parallel (1 batch/core), zero collectives. Per core:
  - k-proj (bf16, et-major single pass) -> kT resident in SBUF
  - v-proj (bf16) in 4 head-quarter passes; quarters 2,3 interleaved into
    early attention beats
  - attention per head-pair pr, sh-outer beats: row-packed score matmuls
    (two K=64 matmuls in row groups 0-1 / 2-3 run concurrently), one exp
    [128,1024] per beat on ScalarE, av accumulation [65,512] per (j, sh)
  - av evacuated to SBUF immediately (psum freed); softmax denominators
    batched per pr: 4 rows -> DRAM -> spread [128,16] -> one reciprocal ->
    DRAM -> partition-broadcast loads -> DVE scale into catT
  - out-proj (bf16 cat x bf16 wo)
PSUM: big 3x[128,1024] (6 banks) + av0/av1 [65,512] (1 bank each) = 8 banks.
DMA: nc.sync = critical path (xk, wk, wq, smalls), nc.scalar = prefetch
(xq, wv, xv, wo[0:3]).
"""

import sys

if "/opt/trn_rl_repo" not in sys.path:
    sys.path.insert(0, "/opt/trn_rl_repo")

import numpy as np

B, S, D, H = 8, 1024, 1024, 16
Dh = D // H  # 64
P = 128
NT = 8
SH = 512

_CACHE = {}


def _prep_x(x):
    # x [S, D] -> [2, 128, 4096]; out[hf, p, k*512 + s'] = x[hf*512+s', k*128+p]
    return np.ascontiguousarray(x.reshape(2, SH, NT, P).transpose(0, 3, 2, 1)).reshape(
        2, P, NT * SH
    )


def _prep_w(Wcat):
    # W [out 1024, in 1024] -> [8, 128, 1024]; out[ot, p, k*128+oc] = W[ot*128+oc, k*128+p]
    return np.ascontiguousarray(Wcat.reshape(NT, P, NT, P).transpose(0, 3, 2, 1)).reshape(
        NT, P, NT * P
    )


def _prep_wv(Wvcat):
    # rhs layout [8, 128, 1024]; out[k, p, e] = Wv_cat[e, k*128+p]
    return np.ascontiguousarray(Wvcat.T.reshape(NT, P, D))


def _prep_bias(b):
    # [1024] -> [128, 8]; out[p, i] = b[i*128+p]
    return np.ascontiguousarray(b.reshape(NT, P).T)


def _bf16(a):
    import ml_dtypes

    return np.asarray(a).astype(ml_dtypes.bfloat16)


def _build():
    import concourse.mybir as mybir
    import concourse.tile as tile
    from concourse import bacc

    dt = mybir.dt
    f32 = dt.float32
    bf16 = dt.bfloat16
    AF = mybir.ActivationFunctionType

    nc = bacc.Bacc(None, target_bir_lowering=False)

    with tile.TileContext(nc) as tc:
        with (
            tc.tile_pool(name="dram", bufs=1, space="DRAM") as dram,
            tc.tile_pool(name="consts", bufs=1) as consts,
            tc.tile_pool(name="xq_p", bufs=1) as xq_p,
            tc.tile_pool(name="xh_p", bufs=2) as xh_p,
            tc.tile_pool(name="wv_p", bufs=2) as wv_p,
            tc.tile_pool(name="wst_p", bufs=3) as wst_p,
            tc.tile_pool(name="kt_p", bufs=1) as kt_p,
            tc.tile_pool(name="vaug_p", bufs=1) as vaug_p,
            tc.tile_pool(name="cat_p", bufs=1) as cat_p,
            tc.tile_pool(name="qp_p", bufs=2) as qp_p,
            tc.tile_pool(name="ex_p", bufs=6) as ex_p,
            tc.tile_pool(name="avst_p", bufs=8) as avst_p,
            tc.tile_pool(name="spr_p", bufs=2) as spr_p,
            tc.tile_pool(name="bcrc_p", bufs=4) as bcrc_p,
            tc.tile_pool(name="tm_p", bufs=2) as tm_p,
            tc.tile_pool(name="st_p", bufs=2) as st_p,
            tc.tile_pool(name="ps", bufs=1, space="PSUM") as ps_p,
        ):
            # ---- DRAM I/O ----
            xq = dram.tile([2, P, NT * SH], bf16, kind="ExternalInput", name="xq", uniquify=False)
            xk = dram.tile([2, P, NT * SH], bf16, kind="ExternalInput", name="xk", uniquify=False)
            xv = dram.tile([2, P, NT * SH], bf16, kind="ExternalInput", name="xv", uniquify=False)
            wq = dram.tile([NT, P, D], bf16, kind="ExternalInput", name="wq", uniquify=False)
            wk = dram.tile([NT, P, D], bf16, kind="ExternalInput", name="wk", uniquify=False)
            wv = dram.tile([NT, P, D], bf16, kind="ExternalInput", name="wv", uniquify=False)
            wo = dram.tile([NT, P, D], bf16, kind="ExternalInput", name="wo", uniquify=False)
            bqd = dram.tile([P, NT], f32, kind="ExternalInput", name="bqd", uniquify=False)
            bkd = dram.tile([P, NT], f32, kind="ExternalInput", name="bkd", uniquify=False)
            bod = dram.tile([P, NT], f32, kind="ExternalInput", name="bod", uniquify=False)
            outT = dram.tile([NT, P, S], bf16, kind="ExternalOutput", name="outT", uniquify=False)

            # ---- consts + persistent SBUF ----
            bq_sb = consts.tile([P, NT], f32, name="bq_sb")
            bk_sb = consts.tile([P, NT], f32, name="bk_sb")
            bo_sb = consts.tile([P, NT], f32, name="bo_sb")
            nc.sync.dma_start(bq_sb[:], bqd[:])
            nc.sync.dma_start(bk_sb[:], bkd[:])
            nc.sync.dma_start(bo_sb[:], bod[:])

            kT = kt_p.tile([P, NT, S], bf16, name="kT")
            catT = cat_p.tile([P, NT, S], bf16, name="catT")
            v_aug = vaug_p.tile([P, NT, H, Dh + 1], bf16, name="v_aug")
            nc.vector.memset(v_aug[:, :, :, Dh], 1.0)

            # dual-queue startup: qSP carries xk0 + odd wk + xq; qAct carries
            # xk1 + even wk + wv + xv. Both queues feed k-proj from t~0.
            wk_tiles = [
                wst_p.tile([P, D], bf16, name="w", tag="w", bufs=NT)
                for _ in range(NT)
            ]
            nc.sync.dma_start(wk_tiles[0][:], wk[0])
            xk0 = xh_p.tile([P, NT * SH], bf16, name="xk0", tag="xh")
            xk1 = xh_p.tile([P, NT * SH], bf16, name="xk1", tag="xh")
            for c in range(4):
                nc.sync.dma_start(
                    xk0[:, c * 1024 : (c + 1) * 1024], xk[0][:, c * 1024 : (c + 1) * 1024]
                )
                nc.scalar.dma_start(
                    xk1[:, c * 1024 : (c + 1) * 1024], xk[1][:, c * 1024 : (c + 1) * 1024]
                )
            for et in range(1, NT):
                eng = nc.sync if et % 2 else nc.scalar
                eng.dma_start(wk_tiles[et][:], wk[et])
            xq_sb = xq_p.tile([P, 2 * NT * SH], bf16, name="xq_sb")
            for c in range(8):
                eng = nc.sync if c % 2 == 0 else nc.scalar
                eng.dma_start(
                    xq_sb[:, c * 1024 : (c + 1) * 1024],
                    xq[c // 4][:, (c % 4) * 1024 : (c % 4 + 1) * 1024],
                )
            wv_sb = [
                wv_p.tile([P, NT * SH], bf16, name=f"wv{eh}", tag="wv")
                for eh in range(2)
            ]
            for k in range(NT):
                nc.scalar.dma_start(
                    wv_sb[0][:, k * SH : (k + 1) * SH], wv[k][:, 0:SH]
                )
            xv0 = xh_p.tile([P, NT * SH], bf16, name="xv0", tag="xh")
            xv1 = xh_p.tile([P, NT * SH], bf16, name="xv1", tag="xh")
            for c in range(8):
                xvt = (xv0, xv1)[c // 4]
                eng = nc.scalar if c % 2 == 0 else nc.sync
                eng.dma_start(
                    xvt[:, (c % 4) * 1024 : (c % 4 + 1) * 1024],
                    xv[c // 4][:, (c % 4) * 1024 : (c % 4 + 1) * 1024],
                )
            for k in range(NT):
                nc.scalar.dma_start(
                    wv_sb[1][:, k * SH : (k + 1) * SH], wv[k][:, SH:D]
                )

            def big_slot():
                return ps_p.tile([P, S], f32, name="bg", tag="big", bufs=3)

            # HAM warmup: PE clock-gate releases only after ~3.4us of sustained
            # matmul activity; without this the whole k-proj runs at 1.2 GHz.
            warm_sb = consts.tile([P, P], bf16, name="warm_sb")
            nc.vector.memset(warm_sb[:], 0.0)

            def emit_warm(n, skip_check=False):
                wps = ps_p.tile([P, SH], f32, name="wps", tag="av0", bufs=1)
                for _ in range(n):
                    nc.tensor.matmul(
                        wps[:, 0:P], warm_sb[:], warm_sb[:],
                        skip_group_check=skip_check,
                    )

            emit_warm(48)

            # ---- k-projection, et-major single pass ----
            # DMA-starve stalls inside the early et groups (xk/wk chunks
            # still landing, consistently ~6.7us in et0 + ~3us in et1-3
            # across runs) exceed the HAM MID window and re-throttle the PE
            # to 1.2 GHz for the rest of k-proj. Interleave data-free warm
            # matmuls INTO the early groups so the PE never idles long
            # enough to re-throttle; they run only while real MMs wait.
            kfill = {0: 4, 1: 3}
            xkh = (xk0, xk1)
            for et in range(NT):
                w = wk_tiles[et]
                slot = big_slot()
                for hf in range(2):
                    for k in range(NT):
                        nc.tensor.matmul(
                            slot[:, hf * SH : (hf + 1) * SH],
                            w[:, k * P : (k + 1) * P],
                            xkh[hf][:, k * SH : (k + 1) * SH],
                            start=(k == 0),
                            stop=(k == NT - 1),
                        )
                        if k % 2 == 1 and k < NT - 1:
                            emit_warm(kfill.get(et, 0), skip_check=True)
                nc.vector.tensor_scalar_add(
                    kT[:, et, :], slot[:], bk_sb[:, et : et + 1]
                )


            # ---- v-projection quarter passes: heads 4q..4q+3 ----
            xvh = (xv0, xv1)

            def emit_vq_chunk(q, tt0, ntt=4):
                eh, qh = divmod(q, 2)
                slot = big_slot()
                for sub in range(ntt):
                    tt = tt0 + sub
                    hf, tl = divmod(tt, 4)
                    for k in range(NT):
                        nc.tensor.matmul(
                            slot[:, sub * 256 : (sub + 1) * 256],
                            xvh[hf][:, k * SH + tl * P : k * SH + (tl + 1) * P],
                            wv_sb[eh][:, k * SH + qh * 256 : k * SH + (qh + 1) * 256],
                            start=(k == 0),
                            stop=(k == NT - 1),
                        )
                    nc.vector.tensor_copy(
                        v_aug[:, tt, q * 4 : (q + 1) * 4, 0:Dh],
                        slot[:, sub * 256 : (sub + 1) * 256].rearrange(
                            "p (g c) -> p g c", c=Dh
                        ),
                    )

            emit_warm(8)
            emit_vq_chunk(0, 0)
            emit_warm(8)
            emit_vq_chunk(0, 4)
            emit_warm(8)
            vq_todo = [(q, tt0) for q in (1, 2, 3) for tt0 in (0, 2, 4, 6)]

            # ---- fused q-projection + attention ----
            def emit_qproj_chunk(pr, idx, qst):
                # 4 matmuls per call; two calls share one psum group so each
                # insert stays under the per-beat tensor slack
                if idx == 0:
                    wqt = wst_p.tile([P, D], bf16, name="wqt", tag="wq", bufs=2)
                    nc.sync.dma_start(wqt[:], wq[pr])
                    qp_t = qp_p.tile([P, S], bf16, name="qp", tag="qp")
                    qst[pr] = [wqt, qp_t, None]
                wqt, qp_t, slot = qst[pr]
                half, kc = divmod(idx, 2)
                hf, ch = divmod(half, 2)
                if kc == 0:
                    slot = big_slot()
                    qst[pr][2] = slot
                for k in range(4 * kc, 4 * kc + 4):
                    nc.tensor.matmul(
                        slot[:, 0:256],
                        wqt[:, k * P : (k + 1) * P],
                        xq_sb[
                            :,
                            hf * 4096 + k * SH + ch * 256 : hf * 4096
                            + k * SH
                            + (ch + 1) * 256,
                        ],
                        start=(k == 0),
                        stop=(k == NT - 1),
                    )
                if kc == 1:
                    nc.vector.tensor_scalar_add(
                        qp_t[:, half * 256 : (half + 1) * 256],
                        slot[:, 0:256],
                        bq_sb[:, pr : pr + 1],
                    )
                return qp_t

            # prefetch all out-projection weights now: qSP is quiet during
            # attention and the slots are fresh (no blocking waits)
            wots = []
            for ft in range(NT):
                wot = wst_p.tile([P, D], bf16, name="wot", tag="wob", bufs=NT)
                nc.sync.dma_start(wot[:], wo[ft])
                wots.append(wot)

            def emit_oproj(ft, sh):
                wot = wots[ft]
                slot = big_slot()
                for et in range(NT):
                    nc.tensor.matmul(
                        slot[:, 0:SH],
                        wot[:, et * P : (et + 1) * P],
                        catT[:, et, sh * SH : (sh + 1) * SH],
                        start=(et == 0),
                        stop=(et == NT - 1),
                    )
                st = st_p.tile([P, SH], bf16, name="st", tag="st", bufs=4)
                nc.vector.tensor_scalar_add(
                    st[:], slot[:, 0:SH], bo_sb[:, ft : ft + 1]
                )
                nc.scalar.dma_start(outT[ft][:, sh * SH : (sh + 1) * SH], st[:])

            qst = {}
            qps = {}
            for idx in range(8):
                qps[0] = emit_qproj_chunk(0, idx, qst)
                if idx in (1, 3, 5):
                    emit_warm(6)

            def emit_sc(pr, qp_t, beat):
                sh, tt = divmod(beat, NT)
                sc = big_slot()
                nc.tensor.matmul(
                    sc[:, 0:SH],
                    kT[0:Dh, pr, tt * P : (tt + 1) * P],
                    qp_t[0:Dh, sh * SH : (sh + 1) * SH],
                )
                nc.tensor.matmul(
                    sc[:, SH:S],
                    kT[Dh:P, pr, tt * P : (tt + 1) * P],
                    qp_t[Dh:P, sh * SH : (sh + 1) * SH],
                )
                ex_t = ex_p.tile([P, S], bf16, name="ex", tag="ex")
                nc.scalar.activation(ex_t[:], sc[:], AF.Exp, scale=0.125)
                return ex_t

            def emit_av(pr, stt, beat, ex_t):
                sh, tt = divmod(beat, NT)
                avs, avsts = stt["avs"], stt["avsts"]
                if tt == 0:
                    for j in range(2):
                        avs[(j, sh)] = ps_p.tile(
                            [Dh + 1, SH], f32, name=f"av{j}", tag=f"av{j}", bufs=1
                        )
                for j in range(2):
                    nc.tensor.matmul(
                        avs[(j, sh)][:],
                        v_aug[:, tt, 2 * pr + j, :],
                        ex_t[:, j * SH : (j + 1) * SH],
                        start=(tt == 0),
                        stop=(tt == NT - 1),
                    )
                if tt == NT - 1:
                    # evacuate to SBUF promptly so the psum bank frees
                    for j in range(2):
                        av = avs.pop((j, sh))
                        avt = avst_p.tile(
                            [Dh + 1, SH], f32, name="avst", tag="avst"
                        )
                        nc.vector.tensor_copy(avt[:], av[:])
                        avsts[(j, sh)] = avt

            def emit_norm(pr, stt, sh):
                avsts = stt["avsts"]
                for j in range(2):
                    avt = avsts.pop((j, sh))
                    # denominator row -> partition 0 (DMA shifts partitions),
                    # broadcast on the idle GpSimd engine, one fast reciprocal
                    # + scale on DVE -- no DRAM round trips
                    dn0 = spr_p.tile([1, SH], f32, name="dn0", tag="spr", bufs=4)
                    nc.sync.dma_start(dn0[0:1, :], avt[Dh : Dh + 1, :])
                    bcrc = bcrc_p.tile([Dh, SH], f32, name="bcrc", tag="bcrc")
                    nc.gpsimd.partition_broadcast(bcrc[:], dn0[0:1, :])
                    nc.vector.reciprocal_approx_fast(bcrc[:], bcrc[:])
                    if j == 0:
                        nc.vector.tensor_mul(
                            catT[0:Dh, pr, sh * SH : (sh + 1) * SH],
                            avt[0:Dh, :],
                            bcrc[:],
                        )
                    else:
                        tm_t = tm_p.tile([Dh, SH], bf16, name="tm", tag="tm")
                        nc.vector.tensor_mul(tm_t[:], avt[0:Dh, :], bcrc[:])
                        nc.sync.dma_start(
                            catT[Dh:P, pr, sh * SH : (sh + 1) * SH], tm_t[:]
                        )

            # seamless beat stream across all head pairs: ACT never sees a
            # pr boundary; av work lags LAG beats behind the score stream
            state = {}
            exs = {}
            LAG = 3
            TOT = NT * 16
            for gb in range(TOT + LAG):
                # AV pair first: following the previous beat's full-row MMs
                # its LDW backgrounds cleanly (216ns slot); emitted after the
                # score pair it pays ~+120ns waiting out the delayed second
                # score MM. The score pair's own first LDW backgrounds fine
                # during AV streams.
                ab = gb - LAG
                if ab >= 0:
                    apr, abeat = divmod(ab, 16)
                    stt = state[apr]
                    emit_av(apr, stt, abeat, exs.pop(ab))
                    if abeat == 7:
                        emit_norm(apr, stt, 0)
                    elif abeat == 15:
                        emit_norm(apr, stt, 1)
                        del state[apr]
                if gb < TOT:
                    pr, beat = divmod(gb, 16)
                    if beat == 0:
                        state[pr] = {"qp": qps.pop(pr), "avs": {}, "avsts": {}}
                    exs[gb] = emit_sc(pr, state[pr]["qp"], beat)
                    if beat % 2 == 0 and pr + 1 < NT:
                        qps[pr + 1] = emit_qproj_chunk(pr + 1, beat // 2, qst)
                    if pr == NT - 1 and beat in (11, 13, 15):
                        emit_oproj((beat - 11) // 2, 0)
                    if pr < 6 and beat in (5, 13) and vq_todo:
                        emit_vq_chunk(*vq_todo.pop(0), ntt=2)

            # ---- output projection, sh-major: sh0 needs no waits, sh1's
            # et7 dependency (pr7 norm) resolves while sh0 runs ----
            for sh in range(2):
                for ft in range(NT):
                    if sh == 0 and ft < 3:
                        continue
                    wot = wots[ft]
                    slot = big_slot()
                    for et in range(NT):
                        nc.tensor.matmul(
                            slot[:, 0:SH],
                            wot[:, et * P : (et + 1) * P],
                            catT[:, et, sh * SH : (sh + 1) * SH],
                            start=(et == 0),
                            stop=(et == NT - 1),
                        )
                    st = st_p.tile([P, SH], bf16, name="st", tag="st", bufs=4)
                    nc.vector.tensor_scalar_add(
                        st[:], slot[:, 0:SH], bo_sb[:, ft : ft + 1]
                    )
                    nc.scalar.dma_start(outT[ft][:, sh * SH : (sh + 1) * SH], st[:])

    nc.compile()
    return nc


def make_in_maps(query, key, value, Wq, Wk, Wv, Wo, bq, bk, bv, bo):
    query = np.asarray(query, np.float32)
    key = np.asarray(key, np.float32)
    value = np.asarray(value, np.float32)
    Wq_c = np.asarray(Wq, np.float32).reshape(D, D)
    Wk_c = np.asarray(Wk, np.float32).reshape(D, D)
    Wv_c = np.asarray(Wv, np.float32).reshape(D, D)
    Wo_c = np.asarray(Wo, np.float32)
    bq_c = np.asarray(bq, np.float32).reshape(D)
    bk_c = np.asarray(bk, np.float32).reshape(D)
    bv_c = np.asarray(bv, np.float32).reshape(D)
    bo_c = np.asarray(bo, np.float32)

    import ml_dtypes

    shared = {
        "wq": _bf16(_prep_w(Wq_c)),
        "wk": _bf16(_prep_w(Wk_c)),
        "wv": _bf16(_prep_wv(Wv_c)),
        "wo": _bf16(_prep_w(Wo_c)),
        "bqd": _prep_bias(bq_c),
        "bkd": _prep_bias(bk_c),
        # attn rows sum to 1: attn @ (v + bv) = attn @ v + bv; bv then flows
        # through the output projection as an extra bias Wo @ bv.
        "bod": _prep_bias(bo_c + Wo_c @ bv_c),
    }
    in_maps = []
    for b in range(B):
        m = dict(shared)
        m["xq"] = _bf16(_prep_x(query[b]))
        m["xk"] = _bf16(_prep_x(key[b]))
        m["xv"] = _bf16(_prep_x(value[b]))
        in_maps.append(m)
    return in_maps


def kernel(query, key, value, mask, Wq, bq, Wk, bk, Wv, bv, Wo, bo):
    from concourse.bass_utils import run_bass_kernel_spmd

    if "nc" not in _CACHE:
        _CACHE["nc"] = _build()
    nc = _CACHE["nc"]

    in_maps = make_in_maps(query, key, value, Wq, Wk, Wv, Wo, bq, bk, bv, bo)
    res = run_bass_kernel_spmd(nc, in_maps, core_ids=list(range(B)))
    out = np.empty((B, S, D), np.float32)
    for b in range(B):
        out[b] = np.asarray(res.results[b]["outT"], np.float32).reshape(D, S).T
    return out

